# revision 43
# baseline (speedup 1.0000x reference)
"""BF15 linear layer for Trainium2, 8-core data-parallel.

Reference semantics:
  y = bf16(bf15(x) @ W.T); y = bf16(fp32(y) + bias)

Strategy:
- Shard x over tokens (32768 -> 8 x 4096), replicate W + bias.
- Host-side prep: x is sliced to its top 16 bits AND bf15-masked (0xFFFE)
  on the host, so the device receives ready-to-multiply bf15-in-bf16 bits
  and needs ZERO preprocessing ops: x stages DMA straight into the resident
  matmul tiles. W is transposed and rounded to bf16 on the host.
- Single bf16 pass: bf15(x) (7 sig bits) is exact in bf16; the only model
  deviation is bf16 rounding of W (~2^-8) and a fused bias add that skips
  the reference's intermediate bf16 rounding. Measured rel_l2 ~ 3.4e-3
  against the fp32 reference (gate is 2e-2).
- Epilogue per group is ONE fused DVE op: y_bf16 = psum_f32 + bias_f32,
  reading PSUM directly (no scalar copy), then the store.
- Filler matmuls on a reserved PSUM bank keep the PE busy (and the HAM
  clock gate open) whenever the schedule predicts a DMA-paced idle
  period; real groups rotate over the other 7 banks and are emitted in
  predicted-arrival order (measured DMA landing times + ~1.2us semaphore
  observation latency), so the PE never starves while inputs stream in.
- Queue split: x stages ride the SP (sync) HWDGE queue; W chunks 0-4 +
  bias ride the ACT (scalar) queue ahead of the y stores; W chunks 5-7
  interleave between x stages on SP. Chunk 0 is split into ko-halves so
  the first matmul group starts as soon as the earliest slice lands
  (PSUM accumulation state survives interleaved fillers, which use a
  different bank).
- Tail: the last group is emitted as a 384+128 column pair and its stores
  go to the idle SP queue, shortening the final DVE+store+semaphore chain
  after the last matmul.
"""

import numpy as np
import ml_dtypes

# Problem shape (hardcoded per contract).
B, S, IN, OUT = 8, 4096, 1024, 4096
N_CORES = 8
M = B * S // N_CORES  # tokens per core = 4096

P = 128
KO = IN // P  # 8 k-subtiles
N_CHUNK = 512
N_CHUNKS = OUT // N_CHUNK  # 8
M_SUB = 128  # tokens per matmul (output partitions)

# x DMA stages (token ranges); the host packs x stage-major so each stage
# transfer is one contiguous 2*KO*sz-byte run per partition (128 large
# descriptors instead of 1024 small ones -> ~2-4us earlier landings).
STAGE_LIST = [(0, 128), (128, 128), (256, 256)] + \
    [(512 + 512 * i, 512) for i in range((B * S // N_CORES - 512) // 512)]

_NC = {}
LAST_RESULTS = None


def _build():
    from concourse import bacc
    import concourse.mybir as mybir
    import concourse.tile as tile
    from concourse.bass import ds, ts

    f32 = mybir.dt.float32
    bf16 = mybir.dt.bfloat16
    u16 = mybir.dt.uint16

    nc = bacc.Bacc("TRN2", target_bir_lowering=False, debug=False,
                   num_devices=N_CORES)
    xt = nc.dram_tensor("xt", [P, KO * M], u16, kind="ExternalInput")
    wt = nc.dram_tensor("wt", [IN, OUT], bf16, kind="ExternalInput")
    bias = nc.dram_tensor("bias", [OUT], f32, kind="ExternalInput")
    y = nc.dram_tensor("y", [M, OUT], bf16, kind="ExternalOutput")

    wr = wt.ap().rearrange("(ko ki) n -> ki ko n", ki=P)  # [128, 8, OUT]
    yr = y.ap()

    stage_off = []
    off = 0
    for _s0, _sz in STAGE_LIST:
        stage_off.append(off)
        off += KO * _sz

    def x_stage_ap(si):
        _s0, _sz = STAGE_LIST[si]
        return xt.ap()[:, stage_off[si]:stage_off[si] + KO * _sz].rearrange(
            "ki (ko m) -> ki ko m", ko=KO)

    # Filler (PE-warm) matmul pacing model: cold rate until the HAM clock
    # gate opens (~3.9us of sustained PE activity), then full rate.
    FILL_COLD_NS, FILL_WARM_NS = 0.427, 0.216  # us per N=512 matmul
    GROUP_NS = 8 * 0.216  # us per 8-matmul group
    PE_T0 = 7.3           # PE free after preamble (us, measured)
    HAM_WARM_T = 11.2     # clock gate opens ~3.9us after fillers start

    # Measured transfer-complete times + ~1.2us DMA-semaphore observation
    # latency (from traces).
    stage_list = STAGE_LIST
    tx_stage = [11.6, 13.2, 16.7, 25.0, 33.0, 40.5, 44.5, 48.5, 52.5, 56.5]
    # tw[5..7] must exceed tx_stage[2..4]: those stages' loads issue the
    # W5-7 DMAs, so no chunk-5..7 group may sort ahead of them.
    tw = [16.5, 22.8, 28.7, 38.9, 44.9, 19.1, 25.2, 33.2]
    TW0A = 14.2  # W0's first ko-half (+ sem latency)
    sub_stage = []   # sub index -> stage index
    tx_sub = []
    for si, (s0, sz) in enumerate(stage_list):
        for _ in range(sz // M_SUB):
            sub_stage.append(si)
            tx_sub.append(tx_stage[si])
    n_subs = len(tx_sub)
    pairs = [(max(tx_sub[sub], tw[c]), sub, c)
             for sub in range(n_subs) for c in range(N_CHUNKS)]
    pairs.sort(key=lambda t: (t[0], t[1], t[2]))
    order = [(sub, c) for _, sub, c in pairs]

    sub_m0 = []
    for si, (s0, sz) in enumerate(stage_list):
        for j in range(sz // M_SUB):
            sub_m0.append(s0 + j * M_SUB)

    with tile.TileContext(nc) as tc:
        with (
            tc.tile_pool(name="const", bufs=1) as const,
            tc.tile_pool(name="brow", bufs=1) as brow,
            tc.tile_pool(name="yout", bufs=24) as yout,
            tc.tile_pool(name="psum", bufs=1, space="PSUM") as psum,
        ):
            wz = const.tile([P, N_CHUNK], bf16, tag="warm")
            nc.vector.memset(wz[:], 0.0)
            pw = psum.tile([P, N_CHUNK], f32, tag="ps7", name="ps7")

            pe_t = [PE_T0]

            def fill_until(t_avail):
                # emit fillers bridging predicted PE idle up to t_avail
                n = 0
                while pe_t[0] + 0.05 < t_avail and n < 64:
                    nc.tensor.matmul(pw[:], wz[:, :P], wz[:],
                                     start=True, stop=True)
                    pe_t[0] += (FILL_COLD_NS if pe_t[0] < HAM_WARM_T
                                else FILL_WARM_NS)
                    n += 1

            # bias first on qAct (tiny), then W chunks 0-4.
            bias_row = brow.tile([1, OUT], f32, tag="brow")
            nc.scalar.dma_start(bias_row[:], bias.ap()[None, :])
            bias_sb = const.tile([P, OUT], f32, tag="bias")
            nc.gpsimd.partition_broadcast(bias_sb[:], bias_row[:])

            w_sb = [const.tile([P, KO, N_CHUNK], bf16, name=f"w{nci}",
                               tag=f"w{nci}") for nci in range(N_CHUNKS)]
            # qAct favors few big transfers (~1us fixed cost per DMA): W0 in
            # ko-halves (early slices start the first groups), W1-4 whole.
            for h in range(2):
                nc.scalar.dma_start(w_sb[0][:, 4 * h:4 * h + 4, :],
                                    wr[:, 4 * h:4 * h + 4, ts(0, N_CHUNK)])
            for nci in (1, 2, 3, 4):
                nc.scalar.dma_start(w_sb[nci][:],
                                    wr[:, :, ts(nci, N_CHUNK)])

            xmm_tiles = [None] * len(stage_list)

            def load_stage(si):
                s0, sz = stage_list[si]
                xmm = const.tile([P, KO, sz], u16, name=f"xmm{si}",
                                 tag=f"xmm{si}")
                nc.sync.dma_start(xmm[:], x_stage_ap(si))
                wq = {2: 5, 3: 6, 4: 7}.get(si)
                if wq is not None:  # W chunks 5..7 interleave on qSP
                    nc.sync.dma_start(w_sb[wq][:],
                                      wr[:, :, ts(wq, N_CHUNK)])
                xmm_tiles[si] = xmm

            loaded = [False] * len(stage_list)
            t_avail = [max(tx_sub[sub], tw[c]) for _, sub, c in pairs]
            for gi, (sub, nci) in enumerate(order):
                si = sub_stage[sub]
                # a chunk-5..7 group needs the stage whose load issues its W
                need = [si, si + 1] + ([nci - 3] if nci >= 5 else [])
                for sj in need:
                    if sj < len(stage_list) and not loaded[sj]:
                        load_stage(sj)
                        loaded[sj] = True
                m0 = sub_m0[sub]
                s0 = stage_list[si][0]
                xmm = xmm_tiles[si]
                ps = psum.tile([P, N_CHUNK], f32, tag=f"ps{gi % 7}",
                               name=f"ps{gi % 7}")
                lhs = xmm[:, :, ds(m0 - s0, M_SUB)].bitcast(bf16)
                if gi == 0:
                    # W0 arrives in ko-halves: start the first group on the
                    # early half, bridge the gap with fillers (accumulation
                    # state lives in the bank, so fillers can interleave).
                    fill_until(TW0A)
                    for ko in range(KO // 2):
                        nc.tensor.matmul(
                            ps[:], lhs[:, ko, :], w_sb[0][:, ko, :],
                            start=(ko == 0), stop=False)
                    pe_t[0] = max(pe_t[0], TW0A) + GROUP_NS / 2
                fill_until(t_avail[gi])
                pe_t[0] = max(pe_t[0], t_avail[gi]) + GROUP_NS
                # Split the final group into a 384+128 pair so the very last
                # epilogue chain (DVE add + store) covers only 128 columns.
                pieces = [(0, 384), (384, 128)] if gi == len(order) - 1 \
                    else [(0, N_CHUNK)]
                for n0, nh in pieces:
                    kos = range(KO // 2, KO) if gi == 0 else range(KO)
                    for ko in kos:
                        nc.tensor.matmul(
                            ps[:, ds(n0, nh)], lhs[:, ko, :],
                            w_sb[nci][:, ko, ds(n0, nh)],
                            start=(ko == 0), stop=(ko == KO - 1))
                    ysb = yout.tile([P, nh], bf16, tag=f"ysb{nh}",
                                    bufs=2 if nh != N_CHUNK else None)
                    # fused epilogue: bf16(psum_f32 + bias_f32), DVE reads PSUM
                    nc.vector.tensor_tensor(
                        ysb[:], ps[:, ds(n0, nh)],
                        bias_sb[:, ds(nci * N_CHUNK + n0, nh)],
                        mybir.AluOpType.add)
                    # route the tail stores to the idle SP queue so the final
                    # store isn't stuck behind the qAct store backlog
                    eng = nc.sync if gi >= len(order) - 4 else nc.scalar
                    eng.dma_start(
                        yr[m0:m0 + M_SUB, ds(nci * N_CHUNK + n0, nh)],
                        ysb[:])
    nc.compile()
    return nc


def _get_nc():
    if "v6" not in _NC:
        _NC["v6"] = _build()
    return _NC["v6"]


def kernel(x: np.ndarray, weight: np.ndarray, bias: np.ndarray) -> np.ndarray:
    from concourse.bass_utils import run_bass_kernel_spmd

    global LAST_RESULTS
    nc = _get_nc()

    x2d = np.ascontiguousarray(x, dtype=np.float32).reshape(B * S, IN)
    # bf15: keep the top 16 bits of each fp32 and clear the last mantissa
    # bit -> exact bf15 value in a bf16 bit pattern (truncation toward zero).
    x2d = ((x2d.view(np.uint32) >> 16) & 0xFFFE).astype(np.uint16)
    wt = np.ascontiguousarray(
        weight.astype(np.float32, copy=False).T.astype(ml_dtypes.bfloat16))
    bias = np.ascontiguousarray(bias, dtype=np.float32)

    def pack_x(shard):  # [M, IN] -> [128, KO*M], stage-major tile image
        blocks = []
        for s0, sz in STAGE_LIST:
            blk = shard[s0:s0 + sz].reshape(sz, KO, P).transpose(2, 1, 0)
            blocks.append(blk.reshape(P, KO * sz))
        return np.ascontiguousarray(np.concatenate(blocks, axis=1))

    in_maps = []
    for c in range(N_CORES):
        in_maps.append({"xt": pack_x(x2d[c * M:(c + 1) * M]),
                        "wt": wt, "bias": bias})

    LAST_RESULTS = run_bass_kernel_spmd(
        nc, in_maps, core_ids=list(range(N_CORES)))
    out = np.concatenate(
        [LAST_RESULTS.results[c]["y"] for c in range(N_CORES)], axis=0)
    return out.reshape(B, S, OUT).astype(ml_dtypes.bfloat16, copy=False)


# revision 48
# speedup vs baseline: 1.0159x; 1.0159x over previous
"""BF15 linear layer for Trainium2, 8-core data-parallel.

Reference semantics:
  y = bf16(bf15(x) @ W.T); y = bf16(fp32(y) + bias)

Strategy:
- Shard x over tokens (32768 -> 8 x 4096), replicate W + bias.
- Host-side prep: x is sliced to its top 16 bits AND bf15-masked (0xFFFE)
  on the host, so the device receives ready-to-multiply bf15-in-bf16 bits
  and needs ZERO preprocessing ops: x stages DMA straight into the resident
  matmul tiles. W is transposed and rounded to bf16 on the host.
- Single bf16 pass: bf15(x) (7 sig bits) is exact in bf16; the only model
  deviation is bf16 rounding of W (~2^-8) and a fused bias add that skips
  the reference's intermediate bf16 rounding. Measured rel_l2 ~ 3.4e-3
  against the fp32 reference (gate is 2e-2).
- Epilogue per group is ONE fused DVE op: y_bf16 = psum_f32 + bias_f32,
  reading PSUM directly (no scalar copy), then the store.
- Filler matmuls on a reserved PSUM bank keep the PE busy (and the HAM
  clock gate open) whenever the schedule predicts a DMA-paced idle
  period; real groups rotate over the other 7 banks and are emitted in
  predicted-arrival order (measured DMA landing times + ~1.2us semaphore
  observation latency), so the PE never starves while inputs stream in.
- Queue split: x stages ride the SP (sync) HWDGE queue; W chunks 0-4 +
  bias ride the ACT (scalar) queue ahead of the y stores; W chunks 5-7
  interleave between x stages on SP. Chunk 0 is split into ko-halves so
  the first matmul group starts as soon as the earliest slice lands
  (PSUM accumulation state survives interleaved fillers, which use a
  different bank).
- Tail: the last group is emitted as a 384+128 column pair and its stores
  go to the idle SP queue, shortening the final DVE+store+semaphore chain
  after the last matmul.
"""

import numpy as np
import ml_dtypes

# Problem shape (hardcoded per contract).
B, S, IN, OUT = 8, 4096, 1024, 4096
N_CORES = 8
M = B * S // N_CORES  # tokens per core = 4096

P = 128
KO = IN // P  # 8 k-subtiles
N_CHUNK = 512
N_CHUNKS = OUT // N_CHUNK  # 8
M_SUB = 128  # tokens per matmul (output partitions)

# x DMA stages (token ranges): small early stages so the first matmul
# groups can start as soon as possible, then 512-token steady stages.
STAGE_LIST = [(0, 128), (128, 128), (256, 256)] + \
    [(512 + 512 * i, 512) for i in range((B * S // N_CORES - 512) // 512)]

_NC = {}
LAST_RESULTS = None


def _build():
    from concourse import bacc
    import concourse.mybir as mybir
    import concourse.tile as tile
    from concourse.bass import ds, ts

    f32 = mybir.dt.float32
    bf16 = mybir.dt.bfloat16
    u16 = mybir.dt.uint16

    nc = bacc.Bacc("TRN2", target_bir_lowering=False, debug=False,
                   num_devices=N_CORES)
    xt = nc.dram_tensor("xt", [IN, M], u16, kind="ExternalInput")
    wt = nc.dram_tensor("wt", [IN, OUT], bf16, kind="ExternalInput")
    bias = nc.dram_tensor("bias", [OUT], f32, kind="ExternalInput")
    y = nc.dram_tensor("y", [M, OUT], bf16, kind="ExternalOutput")

    xr = xt.ap().rearrange("(ko ki) m -> ki ko m", ki=P)  # [128, 8, M]
    wr = wt.ap().rearrange("(ko ki) n -> ki ko n", ki=P)  # [128, 8, OUT]
    yr = y.ap()

    # Filler (PE-warm) matmul pacing model: cold rate until the HAM clock
    # gate opens (~3.9us of sustained PE activity), then full rate.
    FILL_COLD_NS, FILL_WARM_NS = 0.427, 0.216  # us per N=512 matmul
    GROUP_NS = 8 * 0.216  # us per 8-matmul group
    PE_T0 = 7.3           # PE free after preamble (us, measured)
    HAM_WARM_T = 11.2     # clock gate opens ~3.9us after fillers start

    # Measured transfer-complete times + ~1.2us DMA-semaphore observation
    # latency (from traces).
    stage_list = STAGE_LIST
    tx_stage = [12.6, 16.6, 20.6, 30.0, 39.0, 48.9, 54.2, 59.5, 64.8, 70.1]
    # tw[5..7] must exceed tx_stage[2..4]: those stages' loads issue the
    # W5-7 DMAs, so no chunk-5..7 group may sort ahead of them.
    tw = [16.5, 22.8, 28.7, 38.9, 44.9, 25.4, 33.1, 45.1]
    TW0A = 14.2  # W0's first ko-half (+ sem latency)
    sub_stage = []   # sub index -> stage index
    tx_sub = []
    for si, (s0, sz) in enumerate(stage_list):
        for _ in range(sz // M_SUB):
            sub_stage.append(si)
            tx_sub.append(tx_stage[si])
    n_subs = len(tx_sub)
    pairs = [(max(tx_sub[sub], tw[c]), sub, c)
             for sub in range(n_subs) for c in range(N_CHUNKS)]
    pairs.sort(key=lambda t: (t[0], t[1], t[2]))
    order = [(sub, c) for _, sub, c in pairs]

    sub_m0 = []
    for si, (s0, sz) in enumerate(stage_list):
        for j in range(sz // M_SUB):
            sub_m0.append(s0 + j * M_SUB)

    with tile.TileContext(nc) as tc:
        with (
            tc.tile_pool(name="const", bufs=1) as const,
            tc.tile_pool(name="brow", bufs=1) as brow,
            tc.tile_pool(name="yout", bufs=24) as yout,
            tc.tile_pool(name="psum", bufs=1, space="PSUM") as psum,
        ):
            wz = const.tile([P, N_CHUNK], bf16, tag="warm")
            nc.vector.memset(wz[:], 0.0)
            pw = psum.tile([P, N_CHUNK], f32, tag="ps7", name="ps7")

            pe_t = [PE_T0]

            def fill_until(t_avail):
                # emit fillers bridging predicted PE idle up to t_avail
                n = 0
                while pe_t[0] + 0.05 < t_avail and n < 64:
                    nc.tensor.matmul(pw[:], wz[:, :P], wz[:],
                                     start=True, stop=True)
                    pe_t[0] += (FILL_COLD_NS if pe_t[0] < HAM_WARM_T
                                else FILL_WARM_NS)
                    n += 1

            # bias first on qAct (tiny), then W chunks 0-4.
            bias_row = brow.tile([1, OUT], f32, tag="brow")
            nc.scalar.dma_start(bias_row[:], bias.ap()[None, :])
            bias_sb = const.tile([P, OUT], f32, tag="bias")
            nc.gpsimd.partition_broadcast(bias_sb[:], bias_row[:])

            w_sb = [const.tile([P, KO, N_CHUNK], bf16, name=f"w{nci}",
                               tag=f"w{nci}") for nci in range(N_CHUNKS)]
            # qAct favors few big transfers (~1us fixed cost per DMA): W0 in
            # ko-halves (early slices start the first groups), W1-4 whole.
            for h in range(2):
                nc.scalar.dma_start(w_sb[0][:, 4 * h:4 * h + 4, :],
                                    wr[:, 4 * h:4 * h + 4, ts(0, N_CHUNK)])
            for nci in (1, 2, 3, 4):
                nc.scalar.dma_start(w_sb[nci][:],
                                    wr[:, :, ts(nci, N_CHUNK)])

            xmm_tiles = [None] * len(stage_list)

            def load_stage(si):
                s0, sz = stage_list[si]
                xmm = const.tile([P, KO, sz], u16, name=f"xmm{si}",
                                 tag=f"xmm{si}")
                nc.sync.dma_start(xmm[:], xr[:, :, s0:s0 + sz])
                wq = {2: 5, 3: 6, 4: 7}.get(si)
                if wq is not None:  # W chunks 5..7 interleave on qSP
                    nc.sync.dma_start(w_sb[wq][:],
                                      wr[:, :, ts(wq, N_CHUNK)])
                xmm_tiles[si] = xmm

            loaded = [False] * len(stage_list)
            t_avail = [max(tx_sub[sub], tw[c]) for _, sub, c in pairs]
            for gi, (sub, nci) in enumerate(order):
                si = sub_stage[sub]
                # a chunk-5..7 group needs the stage whose load issues its W
                need = [si, si + 1] + ([nci - 3] if nci >= 5 else [])
                for sj in need:
                    if sj < len(stage_list) and not loaded[sj]:
                        load_stage(sj)
                        loaded[sj] = True
                m0 = sub_m0[sub]
                s0 = stage_list[si][0]
                xmm = xmm_tiles[si]
                ps = psum.tile([P, N_CHUNK], f32, tag=f"ps{gi % 7}",
                               name=f"ps{gi % 7}")
                lhs = xmm[:, :, ds(m0 - s0, M_SUB)].bitcast(bf16)
                if gi == 0:
                    # W0 arrives in ko-halves: start the first group on the
                    # early half, bridge the gap with fillers (accumulation
                    # state lives in the bank, so fillers can interleave).
                    fill_until(TW0A)
                    for ko in range(KO // 2):
                        nc.tensor.matmul(
                            ps[:], lhs[:, ko, :], w_sb[0][:, ko, :],
                            start=(ko == 0), stop=False)
                    pe_t[0] = max(pe_t[0], TW0A) + GROUP_NS / 2
                fill_until(t_avail[gi])
                pe_t[0] = max(pe_t[0], t_avail[gi]) + GROUP_NS
                # Split the final group into a 384+128 pair so the very last
                # epilogue chain (DVE add + store) covers only 128 columns.
                pieces = [(0, 384), (384, 128)] if gi == len(order) - 1 \
                    else [(0, N_CHUNK)]
                for n0, nh in pieces:
                    kos = range(KO // 2, KO) if gi == 0 else range(KO)
                    for ko in kos:
                        nc.tensor.matmul(
                            ps[:, ds(n0, nh)], lhs[:, ko, :],
                            w_sb[nci][:, ko, ds(n0, nh)],
                            start=(ko == 0), stop=(ko == KO - 1))
                    ysb = yout.tile([P, nh], bf16, tag=f"ysb{nh}",
                                    bufs=2 if nh != N_CHUNK else None)
                    # fused epilogue: bf16(psum_f32 + bias_f32), DVE reads PSUM
                    nc.vector.tensor_tensor(
                        ysb[:], ps[:, ds(n0, nh)],
                        bias_sb[:, ds(nci * N_CHUNK + n0, nh)],
                        mybir.AluOpType.add)
                    # route the tail stores to the idle SP queue so the final
                    # store isn't stuck behind the qAct store backlog
                    eng = nc.sync if gi >= len(order) - 4 else nc.scalar
                    eng.dma_start(
                        yr[m0:m0 + M_SUB, ds(nci * N_CHUNK + n0, nh)],
                        ysb[:])
    nc.compile()
    return nc


def _get_nc():
    if "v6" not in _NC:
        _NC["v6"] = _build()
    return _NC["v6"]


def kernel(x: np.ndarray, weight: np.ndarray, bias: np.ndarray) -> np.ndarray:
    from concourse.bass_utils import run_bass_kernel_spmd

    global LAST_RESULTS
    nc = _get_nc()

    x2d = np.ascontiguousarray(x, dtype=np.float32).reshape(B * S, IN)
    # bf15: keep the top 16 bits of each fp32 and clear the last mantissa
    # bit -> exact bf15 value in a bf16 bit pattern (truncation toward zero).
    x2d = ((x2d.view(np.uint32) >> 16) & 0xFFFE).astype(np.uint16)
    wt = np.ascontiguousarray(
        weight.astype(np.float32, copy=False).T.astype(ml_dtypes.bfloat16))
    bias = np.ascontiguousarray(bias, dtype=np.float32)

    in_maps = []
    for c in range(N_CORES):
        shard = x2d[c * M:(c + 1) * M]
        in_maps.append({"xt": np.ascontiguousarray(shard.T),
                        "wt": wt, "bias": bias})

    LAST_RESULTS = run_bass_kernel_spmd(
        nc, in_maps, core_ids=list(range(N_CORES)))
    out = np.concatenate(
        [LAST_RESULTS.results[c]["y"] for c in range(N_CORES)], axis=0)
    return out.reshape(B, S, OUT).astype(ml_dtypes.bfloat16, copy=False)
